# revision 4
# baseline (speedup 1.0000x reference)
"""ContextualAttention score kernel for 8 Trainium2 NeuronCores.

Math (per batch): score = softmax_p( 10 * s[p] * y[p,q] ) * mm[p], where
  y[p,q] = sum_{c,di,dj} b_pad[c,pi+di,pj+dj] * f_pad[c,qi+di,qj+dj]
  s[p]   = mm[p] / sqrt(sum(w_p^2) + 1152e-4),  mm[p] = (mask patch == 0)

Sharding: core c -> (batch = c//2, q-half = c%2). No collectives needed
(softmax is over p, which every core holds in full).

On-device layout ("layout B"): out[p, q] with p on partitions.
 - p indexes the 66-wide zero-padded grid flat (pos = 128*t + part,
   t in [0,33)); positions with pos%66 >= 64 are geometric garbage
   (forced to exp(0)=1 and removed by a -128 sum correction + host slice).
 - The 9 patch offsets become 9 shifted matmuls accumulated in PSUM:
   stationary = b_flat[off + 128t : +128] (fp32r), moving = f window
   [128, 8, 64] (fp32r).  fp32r = full-rate fp32 (~1.4e-4 rel err).
 - exp via ScalarE with per-partition scale s10[p]; sum over p via
   ones-matmul; reciprocal + fused (e * mm[p]) * recip on VectorE.
"""

import os
import numpy as np

import concourse.bass as bass
import concourse.bacc as bacc
import concourse.mybir as mybir
import concourse.tile as tile
from concourse import bass_utils

F32 = mybir.dt.float32
F32R = mybir.dt.float32r
AF = mybir.ActivationFunctionType
ALU = mybir.AluOpType

C = 128
H = W = 64
HP = 66                      # padded image width/height
FLAT = HP * HP + 4           # 4360: padded flat + 4 zero tail (AP overrun room)
NPOS = 64 * HP               # 4224 p positions (rows 0..63 of padded grid)
NT = NPOS // 128             # 33 p-tiles
NQC = 4                      # q-chunks per core (each 512 = 8 rows x 64)
FROWS = 34                   # f rows needed per core: 4*8 + 2
FFLAT = FROWS * HP           # 2244
EPS_SUM = 1152e-4            # sum(w*w + 1e-4) = sum(w*w) + 0.1152
SCALE = 10.0
OFFS = [(di, dj) for di in range(3) for dj in range(3)]

LAST_EXEC_NS = None
_CACHE = {}


def _build():
    if "nc" in _CACHE:
        return _CACHE["nc"]
    nc = bacc.Bacc(trn_type="TRN2", target_bir_lowering=False, debug=False)

    bp_d = nc.dram_tensor("bp", [C, FLAT], F32, kind="ExternalInput").ap()
    fp_d = nc.dram_tensor("fp", [C, FFLAT], F32, kind="ExternalInput").ap()
    mp_d = nc.dram_tensor("mp", [1, FLAT], F32, kind="ExternalInput").ap()
    vm_d = nc.dram_tensor("vm", [C, NT], F32, kind="ExternalInput").ap()
    out_d = nc.dram_tensor("out", [NPOS, NQC * 512], F32, kind="ExternalOutput").ap()

    with tile.TileContext(nc) as tc:
        with (
            tc.tile_pool(name="small", bufs=1) as small,
            tc.tile_pool(name="imgtmp", bufs=2) as imgtmp,
            tc.tile_pool(name="imgr", bufs=1) as imgr,
            tc.tile_pool(name="stk", bufs=1) as stk,
            tc.tile_pool(name="rowb", bufs=1) as rowb,
            tc.tile_pool(name="sqp", bufs=2) as sqp,
            tc.tile_pool(name="epool", bufs=40) as epool,
            tc.tile_pool(name="outp", bufs=6) as outp,
            tc.tile_pool(name="recipp", bufs=2) as recipp,
            tc.tile_pool(name="mainps", bufs=4, space="PSUM") as mainps,
            tc.tile_pool(name="dpps", bufs=1, space="PSUM") as dpps,
            tc.tile_pool(name="rowps", bufs=2, space="PSUM") as rowps,
            tc.tile_pool(name="bcps", bufs=1, space="PSUM") as bcps,
        ):
            # ---- constants ----
            ones128_f = small.tile([C, 1], F32, name="ones128_f")
            nc.vector.memset(ones128_f[:, :], 1.0)
            ones128_r = small.tile([C, 1], F32R, name="ones128_r")
            nc.vector.tensor_copy(ones128_r[:, :], ones128_f[:, :])
            ones1_f = small.tile([1, C], F32, name="ones1_f")
            nc.vector.memset(ones1_f[:, :], 1.0)
            ones1_r = small.tile([1, C], F32R, name="ones1_r")
            nc.vector.tensor_copy(ones1_r[:, :], ones1_f[:, :])
            ones9 = small.tile([9, 1], F32, name="ones9")
            nc.vector.memset(ones9[:, :], 1.0)

            vm_sb = small.tile([C, NT], F32, name="vm_sb")
            nc.gpsimd.dma_start(vm_sb[:, :], vm_d[:, :])

            # ---- mask -> mm (patch-sum == 0) ----
            mp_s = rowb.tile([1, FLAT], F32, name="mp_s", tag="rowbuf")
            nc.gpsimd.dma_start(mp_s[:, :], mp_d[:, :])
            mstk = stk.tile([9, NPOS], F32, name="mstk", tag="stack")
            for di in range(3):
                msrc = bass.AP(tensor=mp_s.tensor, offset=mp_s.offset + di * HP,
                               ap=[[FLAT, 1], [1, 3], [1, NPOS]])
                nc.gpsimd.dma_start(mstk[3 * di:3 * di + 3, :], msrc)
            pm_ps = dpps.tile([C, NT], F32, name="pm_ps", tag="dp")
            for t in range(NT):
                nc.tensor.matmul(pm_ps[:, t:t + 1], mstk[:, 128 * t:128 * t + 128],
                                 ones9[:, :], start=True, stop=True)
            mm_raw = small.tile([C, NT], F32, name="mm_raw")
            nc.vector.tensor_scalar(mm_raw[:, :], pm_ps[:, :], 0.0, None,
                                    ALU.is_equal)
            mmv = small.tile([C, NT], F32, name="mmv")
            nc.vector.tensor_mul(mmv[:, :], mm_raw[:, :], vm_sb[:, :])

            # ---- images (fp32 -> fp32r) ----
            b_f32 = imgtmp.tile([C, FLAT], F32, name="b_f32", tag="img")
            nc.gpsimd.dma_start(b_f32[:, :], bp_d[:, :])
            b_r = imgr.tile([C, FLAT], F32R, name="b_r")
            nc.vector.tensor_copy(b_r[:, :], b_f32[:, :])
            f_f32 = imgtmp.tile([C, FFLAT], F32, name="f_f32", tag="img")
            nc.gpsimd.dma_start(f_f32[:, :], fp_d[:, :])
            f_r = imgr.tile([C, FFLAT], F32R, name="f_r")
            nc.vector.tensor_copy(f_r[:, :], f_f32[:, :])

            # ---- denominators: scs[x] = sum_c b[c,x]^2, then 3x3 window ----
            scs_sb = rowb.tile([1, FLAT], F32, name="scs_sb", tag="rowbuf")
            off = 0
            while off < FLAT:
                ln = min(512, FLAT - off)
                sq_t = sqp.tile([C, 512], F32, name="sq_t")
                nc.scalar.activation(sq_t[:, :ln], b_f32[:, off:off + ln],
                                     AF.Square)
                scs_ps = rowps.tile([1, 512], F32, name="scs_ps", tag="row")
                nc.tensor.matmul(scs_ps[0:1, :ln], ones128_f[:, :],
                                 sq_t[:, :ln], start=True, stop=True)
                nc.scalar.copy(scs_sb[0:1, off:off + ln], scs_ps[0:1, :ln])
                off += ln
            sstk = stk.tile([9, NPOS], F32, name="sstk", tag="stack")
            for di in range(3):
                ssrc = bass.AP(tensor=scs_sb.tensor, offset=scs_sb.offset + di * HP,
                               ap=[[FLAT, 1], [1, 3], [1, NPOS]])
                nc.gpsimd.dma_start(sstk[3 * di:3 * di + 3, :], ssrc)
            den_ps = dpps.tile([C, NT], F32, name="den_ps", tag="dp")
            for t in range(NT):
                nc.tensor.matmul(den_ps[:, t:t + 1], sstk[:, 128 * t:128 * t + 128],
                                 ones9[:, :], start=True, stop=True)
            epsb = small.tile([C, 1], F32, name="epsb")
            nc.vector.memset(epsb[:, :], EPS_SUM)
            den_sb = small.tile([C, NT], F32, name="den_sb")
            nc.scalar.activation(den_sb[:, :], den_ps[:, :], AF.Sqrt,
                                 bias=epsb[:, :])
            rden = small.tile([C, NT], F32, name="rden")
            nc.vector.reciprocal(rden[:, :], den_sb[:, :])
            s10 = small.tile([C, NT], F32, name="s10")
            nc.vector.scalar_tensor_tensor(s10[:, :], rden[:, :], SCALE,
                                           mmv[:, :], op0=ALU.mult, op1=ALU.mult)

            # ---- main loop ----
            f_v = f_r.rearrange("c (h w) -> c h w", h=FROWS, w=HP)
            for ch in range(NQC):
                e_tiles = []
                row_ps = rowps.tile([1, 512], F32, name="row_ps", tag="row")
                for t in range(NT):
                    ps = mainps.tile([C, 512], F32, name="ps")
                    for o, (di, dj) in enumerate(OFFS):
                        lo = di * HP + dj + 128 * t
                        nc.tensor.matmul(ps[:, :], b_r[:, lo:lo + 128],
                                         f_v[:, 8 * ch + di:8 * ch + di + 8,
                                             dj:dj + W],
                                         start=(o == 0), stop=(o == 8))
                    e_t = epool.tile([C, 512], F32R, name="e_t")
                    nc.scalar.activation(e_t[:, :], ps[:, :], AF.Exp,
                                         scale=s10[:, t:t + 1])
                    nc.tensor.matmul(row_ps[0:1, :], ones128_r[:, :], e_t[:, :],
                                     start=(t == 0), stop=(t == NT - 1))
                    e_tiles.append(e_t)
                sumrow = small.tile([1, 512], F32R, name="sumrow", tag="sumrow",
                                    bufs=2)
                nc.vector.tensor_scalar(sumrow[0:1, :], row_ps[0:1, :], -128.0,
                                        None, ALU.add)
                bc_ps = bcps.tile([C, 512], F32, name="bc_ps")
                nc.tensor.matmul(bc_ps[:, :], ones1_r[:, :], sumrow[0:1, :],
                                 start=True, stop=True)
                recip = recipp.tile([C, 512], F32, name="recip")
                nc.vector.reciprocal(recip[:, :], bc_ps[:, :])
                for t in range(NT):
                    e_f = e_tiles[t][:, :].bitcast(F32)
                    o_t = outp.tile([C, 512], F32, name="o_t")
                    nc.vector.scalar_tensor_tensor(o_t[:, :], e_f,
                                                   mmv[:, t:t + 1], recip[:, :],
                                                   op0=ALU.mult, op1=ALU.mult)
                    nc.gpsimd.dma_start(
                        out_d[128 * t:128 * t + 128, 512 * ch:512 * ch + 512],
                        o_t[:, :])

    nc.compile()
    _CACHE["nc"] = nc
    return nc


def _prep_inputs(f, b, mask):
    f = np.asarray(f, np.float32)
    b = np.asarray(b, np.float32)
    mask = np.asarray(mask, np.float32)
    B = f.shape[0]

    mask_s = mask[0, 0, ::8, ::8]                       # [64, 64] (batch 0, as ref)
    mp = np.zeros((1, FLAT), np.float32)
    mpv = mp[0, :HP * HP].reshape(HP, HP)
    mpv[1:65, 1:65] = mask_s

    vm = np.zeros((C, NT), np.float32)
    pos = (np.arange(NT)[None, :] * 128 + np.arange(C)[:, None])
    vm[(pos % HP) < 64] = 1.0

    in_maps = []
    for c in range(8):
        bi, h = c // 2, c % 2
        bpad = np.zeros((C, FLAT), np.float32)
        bpv = bpad[:, :HP * HP].reshape(C, HP, HP)
        bpv[:, 1:65, 1:65] = b[bi]
        fpad = np.zeros((C, HP, HP), np.float32)
        fpad[:, 1:65, 1:65] = f[bi]
        fcore = np.ascontiguousarray(
            fpad[:, 32 * h:32 * h + FROWS, :].reshape(C, FFLAT))
        in_maps.append({"bp": bpad, "fp": fcore, "mp": mp, "vm": vm})
    return in_maps


def kernel(f, b, mask):
    global LAST_EXEC_NS
    nc = _build()
    in_maps = _prep_inputs(f, b, mask)
    trace = bool(int(os.environ.get("KBENCH_TRACE", "0")))
    res = bass_utils.run_bass_kernel_spmd(
        nc, in_maps, core_ids=list(range(8)), trace=trace)
    LAST_EXEC_NS = res.exec_time_ns

    B = np.asarray(f).shape[0]
    full = np.empty((B, NPOS // HP * HP // 66 * 66, 0), np.float32)  # unused
    out = np.empty((B, 4096, 4096), np.float32)
    for c in range(8):
        bi, h = c // 2, c % 2
        oc = res.results[c]["out"]                       # [4224, 2048]
        valid = oc.reshape(64, HP, 2048)[:, :64, :].reshape(4096, 2048)
        out[bi, :, 2048 * h:2048 * (h + 1)] = valid
    return out.reshape(B, 4096, 64, 64)
